# revision 17
# baseline (speedup 1.0000x reference)
"""Distributed multi-head attention kernel for Trainium2 (8 NeuronCores).

Problem: B=2, S=2048, D=1024, H=16 heads, DH=64.
  qkv = x @ w_qkv + b_qkv ; per-head softmax(q k^T / 8) v ; out proj.

Sharding (batch x head-group tensor parallel):
  core c = g*4 + j handles batch g and heads 4j..4j+3.

v2 layout: the attention loop is restructured around the ACT (exp)
engine, which is the fixed-function floor (16.8M exps/core @ 1.2GHz
= ~153us).  All projection work (v, q/k pair 1, output projection)
is interleaved into the attention PE stream instead of running in
serial phases, and the AllToAll is split into 4 balanced 256KB
chunks (q columns processed in 128-col stripes chosen so each chunk
carries one 128-col block for EVERY destination core) so collectives
overlap attention instead of serializing at the end.

Per (pr=head-pair, pass) unit: 512 q columns (4 stripes), 16 kt
tiles.  scores are computed transposed (kT stationary) with both
heads packed in the 128-partition contraction; exp output feeds
attn@v as the moving operand with a ones-column in v producing the
softmax row-sums for free.  PSUM budget: psS 2 slots x [128,1024]
(4 banks) + po 3 slots x [128,512] (3 banks) + aux 1 slot (1 bank).
The scalar engine issues no DMAs (each dma_start costs the issuing
queue ~0.6us; exp must never wait).
"""

import numpy as np

import concourse.bacc as bacc
import concourse.mybir as mybir
import concourse.tile as tile
from concourse import bass_utils

F32 = mybir.dt.float32
BF16 = mybir.dt.bfloat16
EXP = mybir.ActivationFunctionType.Exp
MULT = mybir.AluOpType.mult

B, S, D, H = 2, 2048, 1024, 16
DH = D // H            # 64
NCORE = 8
GRP = 4                # cores per batch group
HL = H // GRP          # 4 local heads per core
DTILES = D // 128      # 8 contraction chunks
STILES = S // 128      # 16
SBW = S // NCORE       # 256: per-destination s-block width
VP = 128               # padded v block: [v(64) | ones(1) | zeros(63)]
QW = 512               # q columns per (pr, pass) unit
NPASS = 4              # passes per pr; pass p covers stripes SIG[p]+4g
SIG = [0, 2, 1, 3]     # stripe phase per pass: passes {0,1} cover the
                       # even stripes (col-half 0 of every dest core),
                       # passes {2,3} the odd stripes (col-half 1)

_CACHE = {}
DEBUG = False


def _build():
    nc = bacc.Bacc("TRN2", target_bir_lowering=False, debug=False,
                   num_devices=NCORE)

    xT_d = nc.dram_tensor("xT", [D, S], BF16, kind="ExternalInput")
    wqk_d = nc.dram_tensor("wqk", [D, 2 * HL * DH], BF16, kind="ExternalInput")
    wv_d = nc.dram_tensor("wv", [D, HL * DH], BF16, kind="ExternalInput")
    bqk_d = nc.dram_tensor("bqk", [2 * HL * DH], F32, kind="ExternalInput")
    bv_d = nc.dram_tensor("bv", [HL * DH], F32, kind="ExternalInput")
    wout_d = nc.dram_tensor("wout", [D, D], BF16, kind="ExternalInput")
    bout_d = nc.dram_tensor("bout", [D], F32, kind="ExternalOutput" if False else "ExternalInput")
    out_d = nc.dram_tensor("out", [2 * SBW, D], F32, kind="ExternalOutput")
    if DEBUG:
        dbg = {n: nc.dram_tensor(f"dbg_{n}", shp, BF16, kind="ExternalOutput")
               for n, shp in ([("kT", [128, 2 * S]), ("qp", [128, 4 * S]),
                               ("vext", [128, STILES * HL * VP]),
                               ("aout", [128, 4 * NCORE * 128])] +
                              [(f"a2a_in{k}", [NCORE, 128, 128])
                               for k in range(4)] +
                              [(f"a2a_out{k}", [NCORE, 128, 128])
                               for k in range(4)])}

    groups = [list(range(NCORE))]

    with tile.TileContext(nc) as tc:
        with (
            tc.tile_pool(name="persist", bufs=1) as pers,
            tc.tile_pool(name="big", bufs=DTILES) as big,
            tc.tile_pool(name="wsmall", bufs=1) as wsmall,
            tc.tile_pool(name="ppool", bufs=2) as ppool,
            tc.tile_pool(name="pexpp", bufs=3) as pexpp,
            tc.tile_pool(name="npool", bufs=2) as npool,
            tc.tile_pool(name="fin", bufs=2) as fin,
            tc.tile_pool(name="dram", bufs=1, space="DRAM") as dram,
        ):
            # ---- persistent SBUF tensors ----
            # kT pair tiles: [128, S] per pair pr, partitions 0-63 head
            # 2pr, 64-127 head 2pr+1.  qp: per-head S-column blocks with
            # the other head's partition half zeroed so every scores
            # matmul uses the full 128-row contraction.
            kT = pers.tile([128, 2 * S], BF16, tag="kT")
            qp = pers.tile([128, 4 * S], BF16, tag="qp")
            vext = pers.tile([128, STILES * HL * VP], BF16, tag="vext")
            aout = pers.tile([128, 4 * NCORE * 128], BF16, tag="aout")
            bqk_sb = pers.tile([128, 4], F32, tag="bqk_sb")
            bv_sb = pers.tile([128, HL * DH], F32, tag="bv_sb")
            bv_row = ppool.tile([1, HL * DH], F32, tag="P", name="bv_row")
            bout_sb = pers.tile([128, D], F32, tag="bout_sb")
            bout_row = ppool.tile([1, D], F32, tag="P", name="bout_row")

            wqk_sb = wsmall.tile([128, DTILES * 512], BF16, tag="wqk_sb")
            wv_sb = wsmall.tile([128, DTILES * 256], BF16, tag="wv_sb")
            wout_sb = wsmall.tile([128, DTILES * D], BF16, tag="wout_sb")

            heat_f = wsmall.tile([128, 512], F32, tag="heat_f")
            heat_b = wsmall.tile([128, 512], BF16, tag="heat_b")
            heat_d = dram.tile([128, 512], F32, tag="heat_d", name="heat_d")

            # ---- input DMAs spread over sync/scalar/gpsimd queues (the
            # only DMA-capable engines; scalar is used in the head phase
            # only, so every scalar dma_start drains before the first exp).
            # Small bias tensors go FIRST so they don't queue behind 5MB
            # of x; wout is deferred to mid-attention (gpsimd) so it does
            # not compete with x for HBM bandwidth.
            engs = [nc.sync, nc.scalar, nc.gpsimd]
            nc.sync.dma_start(heat_b[:], wqk_d[0:128, :])
            for et in range(4):
                nc.sync.dma_start(bqk_sb[:, et:et + 1],
                                  bqk_d[et * 128:(et + 1) * 128].unsqueeze(-1))
            nc.scalar.dma_start(bv_row[:], bv_d[:].unsqueeze(0))
            nc.gpsimd.partition_broadcast(bv_sb[:], bv_row[:1, :])
            nc.scalar.dma_start(bout_row[:], bout_d[:].unsqueeze(0))
            nc.gpsimd.partition_broadcast(bout_sb[:], bout_row[:1, :])

            xt_tiles = []
            ei = 0
            for dt in range(DTILES):
                tb = big.tile([128, S], BF16, tag="big", name=f"xt{dt}")
                engs[ei % 3].dma_start(tb[:], xT_d[dt * 128:(dt + 1) * 128, :])
                ei += 1
                engs[ei % 3].dma_start(
                    wqk_sb[:, dt * 512:(dt + 1) * 512],
                    wqk_d[dt * 128:(dt + 1) * 128, :])
                ei += 1
                xt_tiles.append(tb)
            for dt in range(DTILES):
                engs[dt % 3].dma_start(wv_sb[:, dt * 256:(dt + 1) * 256],
                                       wv_d[dt * 128:(dt + 1) * 128, :])

            # warm up the collective stack off the critical path: the first
            # cc op pays ~30us of trigger cost.  Issued AFTER the x/w DMA
            # issues so it does not delay them on the gpsimd queue.
            cc_warm = [dram.tile([NCORE, 2], BF16, tag=f"ccw{i}",
                                 name=f"ccw{i}") for i in range(2)]
            nc.gpsimd.collective_compute(
                "AllToAll", mybir.AluOpType.bypass, replica_groups=groups,
                ins=[cc_warm[0][:].opt()], outs=[cc_warm[1][:].opt()])

            # vext: zero cols 65-127, ones col 64 of each [*,128] block
            nc.vector.memset(
                vext[:].rearrange("p (b w) -> p b w", w=VP)[:, :, DH + 1:VP],
                0.0)
            nc.vector.memset(
                vext[:].rearrange("p (b w) -> p b w", w=VP)[:, :, DH:DH + 1],
                1.0)
            # qp zero halves: head-even blocks zero rows 64-127,
            # head-odd blocks zero rows 0-63
            for pr in range(2):
                nc.vector.memset(qp[64:128, (2 * pr) * S:(2 * pr + 1) * S], 0.0)
                nc.vector.memset(qp[0:64, (2 * pr + 1) * S:(2 * pr + 2) * S], 0.0)

            # ---- PE heater: keeps pstate/HAM up during the DMA ramp ----
            with tc.tile_pool(name="psH", bufs=1, space="PSUM") as psH:
                ph = psH.tile([128, 512], F32, tag="psH")
                for i in range(40):
                    nc.tensor.matmul(ph[:], heat_b[:, 0:128], heat_b[:],
                                     start=True, stop=True)
                nc.vector.tensor_copy(heat_f[:], ph[:])
                nc.sync.dma_start(heat_d[:], heat_f[:])

            # ---- head: q/k pair-0 projection, dt-outer with 4 live
            # accumulators so each x chunk is consumed as its DMA lands ----
            def add_qk(et, sh, acc, width, col0):
                """bias-add acc -> kT/qp.  et 0/1=q pair, 2/3=k pair."""
                if et >= 2:
                    pr = et - 2
                    nc.vector.tensor_scalar_add(
                        kT[:, pr * S + col0:pr * S + col0 + width],
                        acc[:], bqk_sb[:, et:et + 1])
                else:
                    pr = et
                    nc.vector.tensor_scalar_add(
                        qp[0:64, (2 * pr) * S + col0:(2 * pr) * S + col0 + width],
                        acc[0:64, :], bqk_sb[0:64, et:et + 1])
                    nc.vector.tensor_scalar_add(
                        qp[64:128, (2 * pr + 1) * S + col0:
                           (2 * pr + 1) * S + col0 + width],
                        acc[64:128, :], bqk_sb[64:128, et:et + 1])

            with tc.tile_pool(name="psA", bufs=4, space="PSUM") as psA:
                accs = {}
                for et in (2, 0):
                    for sh in range(2):
                        accs[(et, sh)] = psA.tile([128, 1024], F32,
                                                  tag="psA", name=f"qa{et}{sh}")
                for dt in range(DTILES - 1):
                    for et in (2, 0):
                        for sh in range(2):
                            for c in range(2):
                                sl = slice(sh * 1024 + c * 512,
                                           sh * 1024 + (c + 1) * 512)
                                nc.tensor.matmul(
                                    accs[(et, sh)][:, c * 512:(c + 1) * 512],
                                    wqk_sb[:, dt * 512 + et * 128:
                                           dt * 512 + (et + 1) * 128],
                                    xt_tiles[dt][:, sl],
                                    start=(dt == 0), stop=False)
                # last dt tile per accumulator, immediately chased by its
                # bias-add on the (idle) scalar engine so the adds pipeline
                # with the remaining dt7 matmuls instead of serializing
                dt = DTILES - 1
                for et in (2, 0):
                    for sh in range(2):
                        for c in range(2):
                            sl = slice(sh * 1024 + c * 512,
                                       sh * 1024 + (c + 1) * 512)
                            nc.tensor.matmul(
                                accs[(et, sh)][:, c * 512:(c + 1) * 512],
                                wqk_sb[:, dt * 512 + et * 128:
                                       dt * 512 + (et + 1) * 128],
                                xt_tiles[dt][:, sl],
                                start=False, stop=True)
                        acc = accs[(et, sh)]
                        if et >= 2:
                            nc.scalar.add(
                                kT[:, (et - 2) * S + sh * 1024:
                                   (et - 2) * S + sh * 1024 + 1024],
                                acc[:], bqk_sb[:, et:et + 1])
                        else:
                            nc.scalar.add(
                                qp[0:64, (2 * et) * S + sh * 1024:
                                   (2 * et) * S + sh * 1024 + 1024],
                                acc[0:64, :], bqk_sb[0:64, et:et + 1])
                            nc.scalar.add(
                                qp[64:128, (2 * et + 1) * S + sh * 1024:
                                   (2 * et + 1) * S + sh * 1024 + 1024],
                                acc[64:128, :], bqk_sb[64:128, et:et + 1])

            # ---- A2A buffers: 4 chunks of [8 dest, 128 rows, 128 cols] ----
            a2a_in = [dram.tile([NCORE, 128, 128], BF16, tag=f"a2a_in{k}",
                                name=f"a2a_in{k}") for k in range(4)]
            a2a_out = [dram.tile([NCORE, 128, 128], BF16, tag=f"a2a_out{k}",
                                 name=f"a2a_out{k}") for k in range(4)]

            # ---- deferred PE work, drip-fed into the attention loop ----
            # each task: (n_matmuls_done_in_one_call)
            aux_tasks = {p: [] for p in range(8)}  # keyed by global pass idx

            def v_task(st):
                def run(aux_pool):
                    acc = aux_pool.tile([128, HL * DH], F32, tag="aux",
                                        name=f"v{st}")
                    for dt in range(DTILES):
                        nc.tensor.matmul(
                            acc[:],
                            xt_tiles[dt][:, st * 128:(st + 1) * 128],
                            wv_sb[:, dt * 256:(dt + 1) * 256],
                            start=(dt == 0), stop=(dt == DTILES - 1))
                    base = st * HL * VP
                    vv = vext[:, base:base + HL * VP].rearrange(
                        "p (h w) -> p h w", h=HL)
                    nc.vector.tensor_add(
                        vv[:, :, 0:DH],
                        acc[:].rearrange("p (h w) -> p h w", h=HL),
                        bv_sb[:].rearrange("p (h w) -> p h w", h=HL))
                return run

            def qk1_task(et, sh, c):
                def run(aux_pool):
                    acc = aux_pool.tile([128, 512], F32, tag="aux",
                                        name=f"qk1_{et}{sh}{c}")
                    for dt in range(DTILES):
                        nc.tensor.matmul(
                            acc[:],
                            wqk_sb[:, dt * 512 + et * 128:
                                   dt * 512 + (et + 1) * 128],
                            xt_tiles[dt][:, sh * 1024 + c * 512:
                                         sh * 1024 + (c + 1) * 512],
                            start=(dt == 0), stop=(dt == DTILES - 1))
                    add_qk(et, sh, acc, 512, sh * 1024 + c * 512)
                return run

            def outproj_task(pp, gb, c):
                def run(aux_pool):
                    acc = aux_pool.tile([128, 512], F32, tag="aux",
                                        name=f"op{pp}{gb}{c}")
                    for i, (pr_, jr) in enumerate(
                            [(a, b) for a in range(2) for b in range(GRP)]):
                        nc.tensor.matmul(
                            acc[:],
                            aout[:, (pr_ * 2 + pp) * 1024 + (gb * GRP + jr) * 128:
                                 (pr_ * 2 + pp) * 1024 + (gb * GRP + jr + 1) * 128],
                            wout_sb[:, (pr_ * GRP + jr) * D + c * 512:
                                    (pr_ * GRP + jr) * D + (c + 1) * 512],
                            start=(i == 0), stop=(i == 7))
                    res = fin.tile([128, 512], F32, tag="res", name="res")
                    nc.vector.tensor_add(res[:], acc[:],
                                         bout_sb[:, c * 512:(c + 1) * 512])
                    row = gb * SBW + pp * 128
                    nc.sync.dma_start(
                        out_d[row:row + 128, c * 512:(c + 1) * 512], res[:])
                return run

            # v(st) must complete before attn@v consumes vext st=kt at
            # iteration kt+1 of the FIRST pass -> schedule v(st) at slot st.
            for st in range(STILES):
                aux_tasks[0].append(v_task(st))
            qk1s = [qk1_task(et, sh, c)
                    for et in (3, 1) for sh in range(2) for c in range(2)]
            for i, t in enumerate(qk1s):
                aux_tasks[1 + i // 3].append(t)      # passes 1,2,3
            for i, (gb, c) in enumerate(
                    [(g, c) for g in range(2) for c in range(2)]):
                aux_tasks[6 + i // 2].append(outproj_task(0, gb, c))

            # ---- attention: pr x pass x kt, ACT-paced ----
            qpr = qp[:].rearrange("p (hh g four c) -> p hh g four c",
                                  hh=4, four=4, c=128)

            with (
                tc.tile_pool(name="psS", bufs=2, space="PSUM") as psS,
                tc.tile_pool(name="psO", bufs=3, space="PSUM") as psO,
                tc.tile_pool(name="psX", bufs=1, space="PSUM") as psX,
            ):
                for pr in range(2):
                    for p in range(NPASS):
                        gp = pr * NPASS + p
                        tasks = list(aux_tasks[gp])
                        po = [psO.tile([128, QW], F32, tag="psO",
                                       name=f"po{pr}{p}{h}") for h in range(2)]
                        qmov = [qpr[:, 2 * pr + h, :, SIG[p], :]
                                for h in range(2)]
                        prev_p = None
                        for kt in range(STILES):
                            ps2 = psS.tile([128, 1024], F32, tag="psS",
                                           name=f"ps{pr}{p}{kt}")
                            for h in range(2):
                                nc.tensor.matmul(
                                    ps2[:, h * QW:(h + 1) * QW],
                                    kT[:, pr * S + kt * 128:
                                       pr * S + (kt + 1) * 128],
                                    qmov[h], start=True, stop=True)
                            if prev_p is not None:
                                for h in range(2):
                                    vb = ((kt - 1) * HL + 2 * pr + h) * VP
                                    nc.tensor.matmul(
                                        po[h][:],
                                        vext[:, vb:vb + VP],
                                        prev_p[:, h * QW:(h + 1) * QW],
                                        start=(kt - 1 == 0), stop=False)
                            if tasks:
                                tasks.pop(0)(psX)
                            pexp = pexpp.tile([128, 1024], BF16, tag="pexp",
                                              name=f"pexp{pr}{p}{kt}")
                            nc.scalar.activation(pexp[:], ps2[:], EXP,
                                                 scale=0.125)
                            prev_p = pexp
                        for h in range(2):
                            vb = ((STILES - 1) * HL + 2 * pr + h) * VP
                            nc.tensor.matmul(
                                po[h][:],
                                vext[:, vb:vb + VP],
                                prev_p[:, h * QW:(h + 1) * QW],
                                start=False, stop=True)
                        while tasks:
                            tasks.pop(0)(psX)

                        # normalize: softmax row-sums live in po row DH
                        k_idx = pr * 2 + p // 2
                        for h in range(2):
                            rs_row = npool.tile([1, QW], F32, tag="rs_row",
                                                name="rs_row")
                            rs_rec = npool.tile([1, QW], F32, tag="rs_rec",
                                                name="rs_rec")
                            rs_b = npool.tile([64, QW], F32, tag="rs_b",
                                              name="rs_b")
                            attn = npool.tile([64, QW], BF16, tag="attn",
                                              name="attn")
                            nc.vector.tensor_copy(rs_row[:],
                                                  po[h][DH:DH + 1, :])
                            nc.vector.reciprocal_approx_fast(
                                rs_rec[:], rs_row[:1, :])
                            nc.gpsimd.partition_broadcast(rs_b[:],
                                                          rs_rec[:1, :])
                            nc.vector.tensor_tensor(attn[:], po[h][0:DH, :],
                                                    rs_b[:], MULT)
                            dst = a2a_in[k_idx][:].rearrange(
                                "(g par) q c -> par q g c", par=2)[p % 2]
                            nc.sync.dma_start(
                                dst[h * 64:(h + 1) * 64],
                                attn[:].rearrange("q (g c) -> q g c", c=128))
                        if p % 2 == 1:
                            nc.gpsimd.collective_compute(
                                "AllToAll", mybir.AluOpType.bypass,
                                replica_groups=groups,
                                ins=[a2a_in[k_idx][:].opt()],
                                outs=[a2a_out[k_idx][:].opt()])
                            # aout loads are issued on gpsimd AFTER the
                            # collective that produces the LATER of their
                            # two inputs, so they never head-of-line-block
                            # the a2a stripe DMAs on the sync queue.
                            if k_idx == 2:
                                for kk in (0, 2):
                                    nc.gpsimd.dma_start(
                                        aout[:, kk * 1024:(kk + 1) * 1024]
                                        .rearrange("p (j c) -> p j c", c=128),
                                        a2a_out[kk][:].rearrange(
                                            "j p c -> p j c"))
                            if k_idx == 3:
                                for kk in (1, 3):
                                    nc.gpsimd.dma_start(
                                        aout[:, kk * 1024:(kk + 1) * 1024]
                                        .rearrange("p (j c) -> p j c", c=128),
                                        a2a_out[kk][:].rearrange(
                                            "j p c -> p j c"))
                            if k_idx == 0:
                                # wout load: deferred here so it does not
                                # compete with x for HBM at the head
                                for ec in range(DTILES):
                                    nc.gpsimd.dma_start(
                                        wout_sb[:, ec * D:(ec + 1) * D],
                                        wout_d[ec * 128:(ec + 1) * 128, :])

            # ---- tail: output projection col-half 1.  The pr0 half of the
            # contraction (a2a chunk 1, landed long ago) runs while the
            # final AllToAll is in flight; the pr1 half chases it. ----
            with (
                tc.tile_pool(name="psF", bufs=4, space="PSUM") as psF,
                tc.tile_pool(name="psH2", bufs=1, space="PSUM") as psH2,
            ):
                faccs = {}
                for gb in range(2):
                    for c in range(2):
                        facc = psF.tile([128, 512], F32, tag="psF",
                                        name=f"facc{gb}{c}")
                        faccs[(gb, c)] = facc
                        for jr in range(GRP):
                            nc.tensor.matmul(
                                facc[:],
                                aout[:, 1 * 1024 + (gb * GRP + jr) * 128:
                                     1 * 1024 + (gb * GRP + jr + 1) * 128],
                                wout_sb[:, jr * D + c * 512:
                                        jr * D + (c + 1) * 512],
                                start=(jr == 0), stop=False)
                ph2 = psH2.tile([128, 512], F32, tag="psH2")
                for i in range(14):
                    nc.tensor.matmul(ph2[:], heat_b[:, 0:128], heat_b[:],
                                     start=True, stop=True)
                nc.vector.tensor_copy(heat_f[:], ph2[:])
                nc.sync.dma_start(heat_d[:], heat_f[:])
                for gb in range(2):
                    for c in range(2):
                        facc = faccs[(gb, c)]
                        for jr in range(GRP):
                            nc.tensor.matmul(
                                facc[:],
                                aout[:, 3 * 1024 + (gb * GRP + jr) * 128:
                                     3 * 1024 + (gb * GRP + jr + 1) * 128],
                                wout_sb[:, (GRP + jr) * D + c * 512:
                                        (GRP + jr) * D + (c + 1) * 512],
                                start=False, stop=(jr == GRP - 1))
                        res = fin.tile([128, 512], F32, tag="res", name="res")
                        nc.vector.tensor_add(res[:], facc[:],
                                             bout_sb[:, c * 512:(c + 1) * 512])
                        row = gb * SBW + 128
                        nc.sync.dma_start(
                            out_d[row:row + 128, c * 512:(c + 1) * 512],
                            res[:])

            if DEBUG:
                for n, t in [("kT", kT), ("qp", qp), ("vext", vext),
                             ("aout", aout)]:
                    nc.sync.dma_start(dbg[n][:], t[:])
                for k in range(4):
                    nc.sync.dma_start(dbg[f"a2a_in{k}"][:], a2a_in[k][:])
                    nc.sync.dma_start(dbg[f"a2a_out{k}"][:], a2a_out[k][:])

    nc.compile()
    return nc


def _shard(inputs):
    import ml_dtypes
    bf = ml_dtypes.bfloat16
    x = np.asarray(inputs["x"], np.float32)
    w_qkv = np.asarray(inputs["w_qkv"], np.float32)
    b_qkv = np.asarray(inputs["b_qkv"], np.float32)
    w_out = np.asarray(inputs["w_out"], np.float32)
    b_out = np.asarray(inputs["b_out"], np.float32)

    # wout rows permuted to match aout row order: for pair p, rank-in-
    # group jr, t in (0,1): head 4*jr + 2*p + t
    rows = []
    for p in (0, 1):
        for jr in range(GRP):
            for t in (0, 1):
                h = 4 * jr + 2 * p + t
                rows.append(w_out[h * DH:(h + 1) * DH, :])
    wout_perm = np.ascontiguousarray(np.concatenate(rows, 0))

    in_maps = []
    for c in range(NCORE):
        g, j = c // GRP, c % GRP
        cs = slice(j * HL * DH, (j + 1) * HL * DH)
        wqk = np.concatenate([w_qkv[:, :D][:, cs], w_qkv[:, D:2 * D][:, cs]], 1)
        bqk = np.concatenate([b_qkv[:D][cs], b_qkv[D:2 * D][cs]])
        in_maps.append({
            "xT": np.ascontiguousarray(x[g].T).astype(bf),
            "wqk": np.ascontiguousarray(wqk).astype(bf),
            "wv": np.ascontiguousarray(w_qkv[:, 2 * D:][:, cs]).astype(bf),
            "bqk": np.ascontiguousarray(bqk),
            "bv": np.ascontiguousarray(b_qkv[2 * D:][cs]),
            "wout": wout_perm.astype(bf),
            "bout": b_out,
        })
    return in_maps


def _install_ntff_hook():
    """The agent image's antenv lacks axon_hooks; shim it and register the
    ctypes NTFF profiler from trn_agent_boot so trace=True works."""
    import sys
    import types

    if "antenv.axon_hooks" in sys.modules:
        return
    import antenv

    mod = types.ModuleType("antenv.axon_hooks")
    mod._hook = None
    mod.set_axon_ntff_profile_hook = lambda h: setattr(mod, "_hook", h)
    mod.get_axon_ntff_profile_hook = lambda: mod._hook
    sys.modules["antenv.axon_hooks"] = mod
    antenv.axon_hooks = mod
    try:
        from trn_agent_boot.trn_boot import _ntff_profile_via_ctypes
        mod._hook = _ntff_profile_via_ctypes("/opt/axon/libaxon_pjrt.so")
    except Exception as e:  # degrade like upstream: no trace, run still works
        print(f"ntff hook install failed: {e}")


def _run(inputs, trace=False):
    if trace:
        _install_ntff_hook()
    if "nc" not in _CACHE:
        _CACHE["nc"] = _build()
    nc = _CACHE["nc"]
    in_maps = _shard(inputs)
    r = bass_utils.run_bass_kernel_spmd(
        nc, in_maps, core_ids=list(range(NCORE)), trace=trace)
    out = np.empty((B, S, D), np.float32)
    for c in range(NCORE):
        for g in range(B):
            out[g, c * SBW:(c + 1) * SBW, :] = \
                r.results[c]["out"][g * SBW:(g + 1) * SBW]
    return out, r


def kernel(**inputs) -> np.ndarray:
    out, _ = _run(inputs, trace=False)
    return out


# revision 21
# speedup vs baseline: 1.3036x; 1.3036x over previous
"""Distributed multi-head attention kernel for Trainium2 (8 NeuronCores).

Problem: B=2, S=2048, D=1024, H=16 heads, DH=64.
  qkv = x @ w_qkv + b_qkv ; per-head softmax(q k^T / 8) v ; out proj.

Sharding (batch x head-group tensor parallel):
  core c = g*4 + j handles batch g and heads 4j..4j+3.

v2 layout: the attention loop is restructured around the ACT (exp)
engine, which is the fixed-function floor (16.8M exps/core @ 1.2GHz
= ~153us).  All projection work (v, q/k pair 1, output projection)
is interleaved into the attention PE stream instead of running in
serial phases, and the AllToAll is split into 4 balanced 256KB
chunks (q columns processed in 128-col stripes chosen so each chunk
carries one 128-col block for EVERY destination core) so collectives
overlap attention instead of serializing at the end.

Per (pr=head-pair, pass) unit: 512 q columns (4 stripes), 16 kt
tiles.  scores are computed transposed (kT stationary) with both
heads packed in the 128-partition contraction; exp output feeds
attn@v as the moving operand with a ones-column in v producing the
softmax row-sums for free.  PSUM budget: psS 2 slots x [128,1024]
(4 banks) + po 3 slots x [128,512] (3 banks) + aux 1 slot (1 bank).
The scalar engine issues no DMAs (each dma_start costs the issuing
queue ~0.6us; exp must never wait).
"""

import numpy as np

import concourse.bacc as bacc
import concourse.mybir as mybir
import concourse.tile as tile
from concourse import bass_utils

F32 = mybir.dt.float32
BF16 = mybir.dt.bfloat16
EXP = mybir.ActivationFunctionType.Exp
MULT = mybir.AluOpType.mult

B, S, D, H = 2, 2048, 1024, 16
DH = D // H            # 64
NCORE = 8
GRP = 4                # cores per batch group
HL = H // GRP          # 4 local heads per core
DTILES = D // 128      # 8 contraction chunks
STILES = S // 128      # 16
SBW = S // NCORE       # 256: per-destination s-block width
VP = 128               # padded v block: [v(64) | ones(1) | zeros(63)]
QW = 512               # q columns per (pr, pass) unit
NPASS = 4              # passes per pr; pass p covers stripes SIG[p]+4g
SIG = [0, 2, 1, 3]     # stripe phase per pass: passes {0,1} cover the
                       # even stripes (col-half 0 of every dest core),
                       # passes {2,3} the odd stripes (col-half 1)

_CACHE = {}
DEBUG = False


def _build():
    nc = bacc.Bacc("TRN2", target_bir_lowering=False, debug=False,
                   num_devices=NCORE)

    xT_d = nc.dram_tensor("xT", [D, S], BF16, kind="ExternalInput")
    wqk_d = nc.dram_tensor("wqk", [D, 2 * HL * DH], BF16, kind="ExternalInput")
    wv_d = nc.dram_tensor("wv", [D, HL * DH], BF16, kind="ExternalInput")
    bqk_d = nc.dram_tensor("bqk", [2 * HL * DH], F32, kind="ExternalInput")
    bv_d = nc.dram_tensor("bv", [HL * DH], F32, kind="ExternalInput")
    wout_d = nc.dram_tensor("wout", [D, D], BF16, kind="ExternalInput")
    bout_d = nc.dram_tensor("bout", [D], F32, kind="ExternalOutput" if False else "ExternalInput")
    out_d = nc.dram_tensor("out", [2 * SBW, D], F32, kind="ExternalOutput")
    if DEBUG:
        dbg = {n: nc.dram_tensor(f"dbg_{n}", shp, BF16, kind="ExternalOutput")
               for n, shp in ([("kT", [128, 2 * S]), ("qp", [128, 4 * S]),
                               ("vext", [128, STILES * HL * VP]),
                               ("aout", [128, 4 * NCORE * 128])] +
                              [(f"a2a_in{k}", [NCORE, 128, 128])
                               for k in range(4)] +
                              [(f"a2a_out{k}", [NCORE, 128, 128])
                               for k in range(4)])}

    groups = [list(range(NCORE))]

    with tile.TileContext(nc) as tc:
        with (
            tc.tile_pool(name="persist", bufs=1) as pers,
            tc.tile_pool(name="big", bufs=DTILES) as big,
            tc.tile_pool(name="wsmall", bufs=1) as wsmall,
            tc.tile_pool(name="ppool", bufs=2) as ppool,
            tc.tile_pool(name="pexpp", bufs=3) as pexpp,
            tc.tile_pool(name="npool", bufs=2) as npool,
            tc.tile_pool(name="fin", bufs=2) as fin,
            tc.tile_pool(name="dram", bufs=1, space="DRAM") as dram,
        ):
            # ---- persistent SBUF tensors ----
            # kT pair tiles: [128, S] per pair pr, partitions 0-63 head
            # 2pr, 64-127 head 2pr+1.  qp: per-head S-column blocks with
            # the other head's partition half zeroed so every scores
            # matmul uses the full 128-row contraction.
            kT = pers.tile([128, 2 * S], BF16, tag="kT")
            qp = pers.tile([128, 4 * S], BF16, tag="qp")
            vext = pers.tile([128, STILES * HL * VP], BF16, tag="vext")
            aout = pers.tile([128, 4 * NCORE * 128], BF16, tag="aout")
            bqk_sb = pers.tile([128, 4], F32, tag="bqk_sb")
            bv_sb = pers.tile([128, HL * DH], F32, tag="bv_sb")
            bv_row = ppool.tile([1, HL * DH], F32, tag="P", name="bv_row")
            bout_sb = pers.tile([128, D], F32, tag="bout_sb")
            bout_row = ppool.tile([1, D], F32, tag="P", name="bout_row")

            wqk_sb = wsmall.tile([128, DTILES * 512], BF16, tag="wqk_sb")
            wv_sb = wsmall.tile([128, DTILES * 256], BF16, tag="wv_sb")
            wout_sb = wsmall.tile([128, DTILES * D], BF16, tag="wout_sb")

            heat_f = wsmall.tile([128, 512], F32, tag="heat_f")
            heat_b = wsmall.tile([128, 512], BF16, tag="heat_b")
            heat_d = dram.tile([128, 512], F32, tag="heat_d", name="heat_d")

            # ---- input DMAs spread over sync/scalar/gpsimd queues (the
            # only DMA-capable engines; scalar is used in the head phase
            # only, so every scalar dma_start drains before the first exp).
            # Small bias tensors go FIRST so they don't queue behind 5MB
            # of x; wout is deferred to mid-attention (gpsimd) so it does
            # not compete with x for HBM bandwidth.
            # warm up the collective stack: the first cc op pays the
            # barrier + cc-init cost (~30-60us) and OCCUPIES the issuing
            # queue, so it must be the FIRST gpsimd instruction and no
            # input DMA may ride on gpsimd.
            cc_warm = [dram.tile([NCORE, 2], BF16, tag=f"ccw{i}",
                                 name=f"ccw{i}") for i in range(2)]
            nc.gpsimd.collective_compute(
                "AllToAll", mybir.AluOpType.bypass, replica_groups=groups,
                ins=[cc_warm[0][:].opt()], outs=[cc_warm[1][:].opt()])

            engs = [nc.sync, nc.scalar]
            nc.sync.dma_start(heat_b[:], wqk_d[0:128, :])
            for et in range(4):
                nc.sync.dma_start(bqk_sb[:, et:et + 1],
                                  bqk_d[et * 128:(et + 1) * 128].unsqueeze(-1))
            nc.scalar.dma_start(bv_row[:], bv_d[:].unsqueeze(0))
            nc.gpsimd.partition_broadcast(bv_sb[:], bv_row[:1, :])
            nc.scalar.dma_start(bout_row[:], bout_d[:].unsqueeze(0))
            nc.gpsimd.partition_broadcast(bout_sb[:], bout_row[:1, :])

            xt_tiles = []
            ei = 0
            for dt in range(DTILES):
                tb = big.tile([128, S], BF16, tag="big", name=f"xt{dt}")
                engs[ei % 2].dma_start(tb[:], xT_d[dt * 128:(dt + 1) * 128, :])
                ei += 1
                engs[ei % 2].dma_start(
                    wqk_sb[:, dt * 512:(dt + 1) * 512],
                    wqk_d[dt * 128:(dt + 1) * 128, :])
                ei += 1
                xt_tiles.append(tb)
            for dt in range(DTILES):
                engs[dt % 2].dma_start(wv_sb[:, dt * 256:(dt + 1) * 256],
                                       wv_d[dt * 128:(dt + 1) * 128, :])

            # vext: zero cols 65-127, ones col 64 of each [*,128] block
            nc.vector.memset(
                vext[:].rearrange("p (b w) -> p b w", w=VP)[:, :, DH + 1:VP],
                0.0)
            nc.vector.memset(
                vext[:].rearrange("p (b w) -> p b w", w=VP)[:, :, DH:DH + 1],
                1.0)
            # qp zero halves: head-even blocks zero rows 64-127,
            # head-odd blocks zero rows 0-63
            for pr in range(2):
                nc.vector.memset(qp[64:128, (2 * pr) * S:(2 * pr + 1) * S], 0.0)
                nc.vector.memset(qp[0:64, (2 * pr + 1) * S:(2 * pr + 2) * S], 0.0)

            # ---- PE heater: keeps pstate/HAM up during the DMA ramp ----
            with tc.tile_pool(name="psH", bufs=1, space="PSUM") as psH:
                ph = psH.tile([128, 512], F32, tag="psH")
                for i in range(40):
                    nc.tensor.matmul(ph[:], heat_b[:, 0:128], heat_b[:],
                                     start=True, stop=True)
                nc.vector.tensor_copy(heat_f[:], ph[:])
                nc.sync.dma_start(heat_d[:], heat_f[:])

            # ---- head: q/k pair-0 projection, dt-outer with 4 live
            # accumulators so each x chunk is consumed as its DMA lands ----
            def add_qk(et, sh, acc, width, col0):
                """bias-add acc -> kT/qp.  et 0/1=q pair, 2/3=k pair."""
                if et >= 2:
                    pr = et - 2
                    nc.vector.tensor_scalar_add(
                        kT[:, pr * S + col0:pr * S + col0 + width],
                        acc[:], bqk_sb[:, et:et + 1])
                else:
                    pr = et
                    nc.vector.tensor_scalar_add(
                        qp[0:64, (2 * pr) * S + col0:(2 * pr) * S + col0 + width],
                        acc[0:64, :], bqk_sb[0:64, et:et + 1])
                    nc.vector.tensor_scalar_add(
                        qp[64:128, (2 * pr + 1) * S + col0:
                           (2 * pr + 1) * S + col0 + width],
                        acc[64:128, :], bqk_sb[64:128, et:et + 1])

            with tc.tile_pool(name="psA", bufs=4, space="PSUM") as psA:
                accs = {}
                for et in (2, 0):
                    for sh in range(2):
                        accs[(et, sh)] = psA.tile([128, 1024], F32,
                                                  tag="psA", name=f"qa{et}{sh}")
                for dt in range(DTILES - 1):
                    for et in (2, 0):
                        for sh in range(2):
                            for c in range(2):
                                sl = slice(sh * 1024 + c * 512,
                                           sh * 1024 + (c + 1) * 512)
                                nc.tensor.matmul(
                                    accs[(et, sh)][:, c * 512:(c + 1) * 512],
                                    wqk_sb[:, dt * 512 + et * 128:
                                           dt * 512 + (et + 1) * 128],
                                    xt_tiles[dt][:, sl],
                                    start=(dt == 0), stop=False)
                # last dt tile per accumulator, immediately chased by its
                # bias-add on the (idle) scalar engine so the adds pipeline
                # with the remaining dt7 matmuls instead of serializing
                dt = DTILES - 1
                for et in (2, 0):
                    for sh in range(2):
                        for c in range(2):
                            sl = slice(sh * 1024 + c * 512,
                                       sh * 1024 + (c + 1) * 512)
                            nc.tensor.matmul(
                                accs[(et, sh)][:, c * 512:(c + 1) * 512],
                                wqk_sb[:, dt * 512 + et * 128:
                                       dt * 512 + (et + 1) * 128],
                                xt_tiles[dt][:, sl],
                                start=False, stop=True)
                        acc = accs[(et, sh)]
                        if et >= 2:
                            nc.scalar.add(
                                kT[:, (et - 2) * S + sh * 1024:
                                   (et - 2) * S + sh * 1024 + 1024],
                                acc[:], bqk_sb[:, et:et + 1])
                        else:
                            nc.scalar.add(
                                qp[0:64, (2 * et) * S + sh * 1024:
                                   (2 * et) * S + sh * 1024 + 1024],
                                acc[0:64, :], bqk_sb[0:64, et:et + 1])
                            nc.scalar.add(
                                qp[64:128, (2 * et + 1) * S + sh * 1024:
                                   (2 * et + 1) * S + sh * 1024 + 1024],
                                acc[64:128, :], bqk_sb[64:128, et:et + 1])

            # ---- A2A buffers: 4 chunks of [8 dest, 128 rows, 128 cols] ----
            a2a_in = [dram.tile([NCORE, 128, 128], BF16, tag=f"a2a_in{k}",
                                name=f"a2a_in{k}") for k in range(4)]
            a2a_out = [dram.tile([NCORE, 128, 128], BF16, tag=f"a2a_out{k}",
                                 name=f"a2a_out{k}") for k in range(4)]

            # ---- deferred PE work, drip-fed into the attention loop ----
            # each task: (n_matmuls_done_in_one_call)
            aux_tasks = {p: [] for p in range(8)}  # keyed by global pass idx

            def v_task(st):
                def run(aux_pool):
                    acc = aux_pool.tile([128, HL * DH], F32, tag="aux",
                                        name=f"v{st}")
                    for dt in range(DTILES):
                        nc.tensor.matmul(
                            acc[:],
                            xt_tiles[dt][:, st * 128:(st + 1) * 128],
                            wv_sb[:, dt * 256:(dt + 1) * 256],
                            start=(dt == 0), stop=(dt == DTILES - 1))
                    base = st * HL * VP
                    vv = vext[:, base:base + HL * VP].rearrange(
                        "p (h w) -> p h w", h=HL)
                    nc.vector.tensor_add(
                        vv[:, :, 0:DH],
                        acc[:].rearrange("p (h w) -> p h w", h=HL),
                        bv_sb[:].rearrange("p (h w) -> p h w", h=HL))
                return run

            def qk1_task(et, sh, c):
                def run(aux_pool):
                    acc = aux_pool.tile([128, 512], F32, tag="aux",
                                        name=f"qk1_{et}{sh}{c}")
                    for dt in range(DTILES):
                        nc.tensor.matmul(
                            acc[:],
                            wqk_sb[:, dt * 512 + et * 128:
                                   dt * 512 + (et + 1) * 128],
                            xt_tiles[dt][:, sh * 1024 + c * 512:
                                         sh * 1024 + (c + 1) * 512],
                            start=(dt == 0), stop=(dt == DTILES - 1))
                    add_qk(et, sh, acc, 512, sh * 1024 + c * 512)
                return run

            def outproj_task(pp, gb, c):
                def run(aux_pool):
                    acc = aux_pool.tile([128, 512], F32, tag="aux",
                                        name=f"op{pp}{gb}{c}")
                    for i, (pr_, jr) in enumerate(
                            [(a, b) for a in range(2) for b in range(GRP)]):
                        nc.tensor.matmul(
                            acc[:],
                            aout[:, (pr_ * 2 + pp) * 1024 + (gb * GRP + jr) * 128:
                                 (pr_ * 2 + pp) * 1024 + (gb * GRP + jr + 1) * 128],
                            wout_sb[:, (pr_ * GRP + jr) * D + c * 512:
                                    (pr_ * GRP + jr) * D + (c + 1) * 512],
                            start=(i == 0), stop=(i == 7))
                    res = fin.tile([128, 512], F32, tag="res", name="res")
                    nc.vector.tensor_add(res[:], acc[:],
                                         bout_sb[:, c * 512:(c + 1) * 512])
                    row = gb * SBW + pp * 128
                    nc.sync.dma_start(
                        out_d[row:row + 128, c * 512:(c + 1) * 512], res[:])
                return run

            # v(st) must complete before attn@v consumes vext st=kt at
            # iteration kt+1 of the FIRST pass -> schedule v(st) at slot st.
            for st in range(STILES):
                aux_tasks[0].append(v_task(st))
            qk1s = [qk1_task(et, sh, c)
                    for et in (3, 1) for sh in range(2) for c in range(2)]
            for i, t in enumerate(qk1s):
                aux_tasks[1 + i // 3].append(t)      # passes 1,2,3
            # pp0 outproj needs aout chunk 2 (lands ~end of pass 6), so all
            # four chunks go in pass 7
            for gb in range(2):
                for c in range(2):
                    aux_tasks[7].append(outproj_task(0, gb, c))

            # ---- attention: pr x pass x kt, ACT-paced ----
            qpr = qp[:].rearrange("p (hh g four c) -> p hh g four c",
                                  hh=4, four=4, c=128)

            with (
                tc.tile_pool(name="psS", bufs=2, space="PSUM") as psS,
                tc.tile_pool(name="psO", bufs=3, space="PSUM") as psO,
                tc.tile_pool(name="psX", bufs=1, space="PSUM") as psX,
            ):
                for pr in range(2):
                    for p in range(NPASS):
                        gp = pr * NPASS + p
                        tasks = list(aux_tasks[gp])
                        po = [psO.tile([128, QW], F32, tag="psO",
                                       name=f"po{pr}{p}{h}") for h in range(2)]
                        qmov = [qpr[:, 2 * pr + h, :, SIG[p], :]
                                for h in range(2)]
                        prev_p = None
                        for kt in range(STILES):
                            ps2 = psS.tile([128, 1024], F32, tag="psS",
                                           name=f"ps{pr}{p}{kt}")
                            for h in range(2):
                                nc.tensor.matmul(
                                    ps2[:, h * QW:(h + 1) * QW],
                                    kT[:, pr * S + kt * 128:
                                       pr * S + (kt + 1) * 128],
                                    qmov[h], start=True, stop=True)
                            if prev_p is not None:
                                for h in range(2):
                                    vb = ((kt - 1) * HL + 2 * pr + h) * VP
                                    nc.tensor.matmul(
                                        po[h][:],
                                        vext[:, vb:vb + VP],
                                        prev_p[:, h * QW:(h + 1) * QW],
                                        start=(kt - 1 == 0), stop=False)
                            if tasks:
                                tasks.pop(0)(psX)
                            pexp = pexpp.tile([128, 1024], BF16, tag="pexp",
                                              name=f"pexp{pr}{p}{kt}")
                            nc.scalar.activation(pexp[:], ps2[:], EXP,
                                                 scale=0.125)
                            prev_p = pexp
                        for h in range(2):
                            vb = ((STILES - 1) * HL + 2 * pr + h) * VP
                            nc.tensor.matmul(
                                po[h][:],
                                vext[:, vb:vb + VP],
                                prev_p[:, h * QW:(h + 1) * QW],
                                start=False, stop=True)
                        while tasks:
                            tasks.pop(0)(psX)

                        # normalize: softmax row-sums live in po row DH
                        k_idx = pr * 2 + p // 2
                        if gp == 6:
                            # aout loads for the pp0 output projection;
                            # issued here (gpsimd, post-collective) so they
                            # never head-of-line-block stripe DMAs
                            for kk in (0, 2):
                                nc.gpsimd.dma_start(
                                    aout[:, kk * 1024:(kk + 1) * 1024]
                                    .rearrange("p (j c) -> p j c", c=128),
                                    a2a_out[kk][:].rearrange("j p c -> p j c"))
                        for h in range(2):
                            rs_row = npool.tile([1, QW], F32, tag="rs_row",
                                                name="rs_row")
                            rs_rec = npool.tile([1, QW], F32, tag="rs_rec",
                                                name="rs_rec")
                            rs_b = npool.tile([64, QW], F32, tag="rs_b",
                                              name="rs_b")
                            attn = npool.tile([64, QW], BF16, tag="attn",
                                              name="attn")
                            nc.vector.tensor_copy(rs_row[:],
                                                  po[h][DH:DH + 1, :])
                            nc.vector.reciprocal_approx_fast(
                                rs_rec[:], rs_row[:1, :])
                            nc.gpsimd.partition_broadcast(rs_b[:],
                                                          rs_rec[:1, :])
                            nc.vector.tensor_tensor(attn[:], po[h][0:DH, :],
                                                    rs_b[:], MULT)
                            dst = a2a_in[k_idx][:].rearrange(
                                "(g par) q c -> par q g c", par=2)[p % 2]
                            nc.sync.dma_start(
                                dst[h * 64:(h + 1) * 64],
                                attn[:].rearrange("q (g c) -> q g c", c=128))
                        if p % 2 == 1:
                            nc.gpsimd.collective_compute(
                                "AllToAll", mybir.AluOpType.bypass,
                                replica_groups=groups,
                                ins=[a2a_in[k_idx][:].opt()],
                                outs=[a2a_out[k_idx][:].opt()])
                            if k_idx == 3:
                                for kk in (1, 3):
                                    nc.gpsimd.dma_start(
                                        aout[:, kk * 1024:(kk + 1) * 1024]
                                        .rearrange("p (j c) -> p j c", c=128),
                                        a2a_out[kk][:].rearrange(
                                            "j p c -> p j c"))
                            if k_idx == 0:
                                # wout load: deferred here so it does not
                                # compete with x for HBM at the head
                                for ec in range(DTILES):
                                    nc.gpsimd.dma_start(
                                        wout_sb[:, ec * D:(ec + 1) * D],
                                        wout_d[ec * 128:(ec + 1) * 128, :])

            # ---- tail: output projection col-half 1.  The pr0 half of the
            # contraction (a2a chunk 1, landed long ago) runs while the
            # final AllToAll is in flight; the pr1 half chases it. ----
            with (
                tc.tile_pool(name="psF", bufs=4, space="PSUM") as psF,
                tc.tile_pool(name="psH2", bufs=1, space="PSUM") as psH2,
            ):
                faccs = {}
                for gb in range(2):
                    for c in range(2):
                        facc = psF.tile([128, 512], F32, tag="psF",
                                        name=f"facc{gb}{c}")
                        faccs[(gb, c)] = facc
                        for jr in range(GRP):
                            nc.tensor.matmul(
                                facc[:],
                                aout[:, 1 * 1024 + (gb * GRP + jr) * 128:
                                     1 * 1024 + (gb * GRP + jr + 1) * 128],
                                wout_sb[:, jr * D + c * 512:
                                        jr * D + (c + 1) * 512],
                                start=(jr == 0), stop=False)
                ph2 = psH2.tile([128, 512], F32, tag="psH2")
                for i in range(14):
                    nc.tensor.matmul(ph2[:], heat_b[:, 0:128], heat_b[:],
                                     start=True, stop=True)
                nc.vector.tensor_copy(heat_f[:], ph2[:])
                nc.sync.dma_start(heat_d[:], heat_f[:])
                for gb in range(2):
                    for c in range(2):
                        facc = faccs[(gb, c)]
                        for jr in range(GRP):
                            nc.tensor.matmul(
                                facc[:],
                                aout[:, 3 * 1024 + (gb * GRP + jr) * 128:
                                     3 * 1024 + (gb * GRP + jr + 1) * 128],
                                wout_sb[:, (GRP + jr) * D + c * 512:
                                        (GRP + jr) * D + (c + 1) * 512],
                                start=False, stop=(jr == GRP - 1))
                        res = fin.tile([128, 512], F32, tag="res", name="res")
                        nc.vector.tensor_add(res[:], facc[:],
                                             bout_sb[:, c * 512:(c + 1) * 512])
                        row = gb * SBW + 128
                        nc.sync.dma_start(
                            out_d[row:row + 128, c * 512:(c + 1) * 512],
                            res[:])

            if DEBUG:
                for n, t in [("kT", kT), ("qp", qp), ("vext", vext),
                             ("aout", aout)]:
                    nc.sync.dma_start(dbg[n][:], t[:])
                for k in range(4):
                    nc.sync.dma_start(dbg[f"a2a_in{k}"][:], a2a_in[k][:])
                    nc.sync.dma_start(dbg[f"a2a_out{k}"][:], a2a_out[k][:])

    nc.compile()
    return nc


def _shard(inputs):
    import ml_dtypes
    bf = ml_dtypes.bfloat16
    x = np.asarray(inputs["x"], np.float32)
    w_qkv = np.asarray(inputs["w_qkv"], np.float32)
    b_qkv = np.asarray(inputs["b_qkv"], np.float32)
    w_out = np.asarray(inputs["w_out"], np.float32)
    b_out = np.asarray(inputs["b_out"], np.float32)

    # wout rows permuted to match aout row order: for pair p, rank-in-
    # group jr, t in (0,1): head 4*jr + 2*p + t
    rows = []
    for p in (0, 1):
        for jr in range(GRP):
            for t in (0, 1):
                h = 4 * jr + 2 * p + t
                rows.append(w_out[h * DH:(h + 1) * DH, :])
    wout_perm = np.ascontiguousarray(np.concatenate(rows, 0))

    in_maps = []
    for c in range(NCORE):
        g, j = c // GRP, c % GRP
        cs = slice(j * HL * DH, (j + 1) * HL * DH)
        wqk = np.concatenate([w_qkv[:, :D][:, cs], w_qkv[:, D:2 * D][:, cs]], 1)
        bqk = np.concatenate([b_qkv[:D][cs], b_qkv[D:2 * D][cs]])
        in_maps.append({
            "xT": np.ascontiguousarray(x[g].T).astype(bf),
            "wqk": np.ascontiguousarray(wqk).astype(bf),
            "wv": np.ascontiguousarray(w_qkv[:, 2 * D:][:, cs]).astype(bf),
            "bqk": np.ascontiguousarray(bqk),
            "bv": np.ascontiguousarray(b_qkv[2 * D:][cs]),
            "wout": wout_perm.astype(bf),
            "bout": b_out,
        })
    return in_maps


def _install_ntff_hook():
    """The agent image's antenv lacks axon_hooks; shim it and register the
    ctypes NTFF profiler from trn_agent_boot so trace=True works."""
    import sys
    import types

    if "antenv.axon_hooks" in sys.modules:
        return
    import antenv

    mod = types.ModuleType("antenv.axon_hooks")
    mod._hook = None
    mod.set_axon_ntff_profile_hook = lambda h: setattr(mod, "_hook", h)
    mod.get_axon_ntff_profile_hook = lambda: mod._hook
    sys.modules["antenv.axon_hooks"] = mod
    antenv.axon_hooks = mod
    try:
        from trn_agent_boot.trn_boot import _ntff_profile_via_ctypes
        mod._hook = _ntff_profile_via_ctypes("/opt/axon/libaxon_pjrt.so")
    except Exception as e:  # degrade like upstream: no trace, run still works
        print(f"ntff hook install failed: {e}")


def _run(inputs, trace=False):
    if trace:
        _install_ntff_hook()
    if "nc" not in _CACHE:
        _CACHE["nc"] = _build()
    nc = _CACHE["nc"]
    in_maps = _shard(inputs)
    r = bass_utils.run_bass_kernel_spmd(
        nc, in_maps, core_ids=list(range(NCORE)), trace=trace)
    out = np.empty((B, S, D), np.float32)
    for c in range(NCORE):
        for g in range(B):
            out[g, c * SBW:(c + 1) * SBW, :] = \
                r.results[c]["out"][g * SBW:(g + 1) * SBW]
    return out, r


def kernel(**inputs) -> np.ndarray:
    out, _ = _run(inputs, trace=False)
    return out
